# revision 20
# baseline (speedup 1.0000x reference)
"""Trainium2 Bass kernel for nn_NeuralNetwork_89833535963626.

Computes, for x of shape [N, 1] and a tiny 1-10-1 MLP:
    xw  = mod(x + pi, 2*pi) - pi
    out = tanh(xw @ w1.T + b1) @ w2.T + b2

Sharding: pure data parallel over 8 NeuronCores (batch split).
Weights are baked into the instruction stream at build time.

Per-core pipeline (all engines concurrent under Tile):
  DVE : angle wrap via magic-number rounding + split-constant residual;
        the wrap-count decision is made with Sterbenz-exact quantities so
        boundary elements match IEEE f32 floored-mod bit-for-bit.
  ACT : 10x tanh(scale*r + bias) with w1/b1 (and the -pi shift) folded
        into the activation's free affine.
  PE  : weighted sum over the 10 hidden units as diag(w2_j) matmuls
        accumulated in PSUM (fp32).
  DVE : PSUM -> SBUF evacuation fused with the +b2 bias.
"""
import functools
import sys

import numpy as np

for _p in ("/opt/trn_rl_repo", "/root/.axon_site", "/root/.axon_site/_ro/pypackages"):
    if _p not in sys.path:
        sys.path.append(_p)

from contextlib import ExitStack

import concourse.bass as bass
import concourse.tile as tile
from concourse import bacc, mybir
from concourse.bass_utils import run_bass_kernel_spmd

AF = mybir.ActivationFunctionType
OP = mybir.AluOpType
DT = mybir.dt

f32 = np.float32

N_TOTAL = 4194304
N_CORES = 8
N_CORE = N_TOTAL // N_CORES
P = 128
FD = 2048                  # legacy default tile free-dim
FD_LIST = (512, 1024, 1024, 1024, 512)   # per-supertile free dims; sum*P == N_CORE
CHUNK = 512                # psum / matmul chunk (one PSUM bank, fp32)
HID = 10
MM_MODE = "f32"            # "f32" | "f32r" | "bf16" — PE operand dtype
GP_OFFLOAD = False

# wrap constants
PI = f32(np.pi)
B = f32(2.0 * np.pi)
_C_HI = (np.float32(B).view(np.uint32) & np.uint32(0xFFFFE000)).view(np.float32)
C_HI = f32(_C_HI)                       # 11-bit mantissa: k*C_HI exact for |k|<=8192
C_LO = f32(np.float64(B) - np.float64(C_HI))
INV_B = f32(1.0 / np.float64(B))
MAGIC = f32(2 ** 23 + 2 ** 22)


def emit(nc, tc, x_dram, y_dram, w1, b1, w2, b2, fd_list, mm_mode=MM_MODE,
         gp_offload=True, pe_js=HID):
    h_dt = {"bf16": DT.bfloat16, "f32r": DT.float32r, "f32": DT.float32}[mm_mode]
    diag_dt = h_dt

    def mm_ap(ap):
        return ap

    # engine for the six single-input tensor_scalar ops of the wrap chain
    ts_eng = nc.gpsimd if gp_offload else nc.vector

    ctx = ExitStack()
    with ctx:
        const = ctx.enter_context(tc.tile_pool(name="const", bufs=1))
        xp = ctx.enter_context(tc.tile_pool(name="xp", bufs=2))
        wp = ctx.enter_context(tc.tile_pool(name="wrap", bufs=1))
        rp = ctx.enter_context(tc.tile_pool(name="rp", bufs=2))
        hp = ctx.enter_context(tc.tile_pool(name="hp", bufs=3))
        op_ = ctx.enter_context(tc.tile_pool(name="op", bufs=2))
        pp = ctx.enter_context(tc.tile_pool(name="pp", bufs=8, space="PSUM"))

        # diag(w2_j) stationary tiles, synthesized on-chip
        iota_t = const.tile([P, P], DT.int32, tag="iota", name="iota_t")
        nc.gpsimd.iota(iota_t[:], pattern=[[1, P]], base=0, channel_multiplier=-1)
        ident = const.tile([P, P], DT.float32, tag="ident", name="ident")
        nc.vector.tensor_scalar(ident[:], iota_t[:], 0, None, OP.is_equal)
        diags = []
        for j in range(HID):
            dj = const.tile([P, P], diag_dt, tag=f"diag{j}", name=f"diag{j}")
            nc.vector.tensor_scalar(dj[:], ident[:], float(w2[0, j]), None, OP.mult)
            diags.append(dj)
        biases = []
        for j in range(HID):
            bval = float(f32(np.float64(b1[j]) - np.float64(w1[j, 0]) * np.float64(PI)))
            bt = const.tile([P, 1], DT.float32, tag=f"b{j}", name=f"bias{j}")
            nc.gpsimd.memset(bt[:], bval)
            biases.append(bt)

        x_flat = x_dram.ap()
        y_flat = y_dram.ap()

        off = 0
        for t, fd in enumerate(fd_list):
            nch = fd // CHUNK
            x_src = x_flat[off:off + P * fd].rearrange("(p f) -> p f", f=fd)
            y_dst = y_flat[off:off + P * fd].rearrange("(p f) -> p f", f=fd)
            off += P * fd

            xt = xp.tile([P, fd], DT.float32, tag="x", name=f"x{t}")
            nc.sync.dma_start(xt[:], x_src)

            tt = wp.tile([P, fd], DT.float32, tag="t", name=f"t{t}")
            ts_eng.tensor_scalar(tt[:], xt[:], float(PI), None, OP.add)
            km = wp.tile([P, fd], DT.float32, tag="km", name=f"km{t}")
            ts_eng.tensor_scalar(km[:], xt[:], float(INV_B), 0.5, OP.mult, OP.add)
            uu = wp.tile([P, fd], DT.float32, tag="u", name=f"u{t}")
            ts_eng.tensor_scalar(uu[:], km[:], float(MAGIC), float(MAGIC), OP.add, OP.subtract)
            s1 = wp.tile([P, fd], DT.float32, tag="s1", name=f"s1{t}")
            nc.vector.scalar_tensor_tensor(s1[:], uu[:], float(-C_HI), tt[:], OP.mult, OP.add)
            ucl = wp.tile([P, fd], DT.float32, tag="ucl", name=f"ucl{t}")
            ts_eng.tensor_scalar(ucl[:], uu[:], float(C_LO), None, OP.mult)
            mhi = wp.tile([P, fd], DT.float32, tag="mhi", name=f"mhi{t}")
            nc.vector.tensor_tensor(mhi[:], s1[:], ucl[:], OP.is_ge)
            mh1 = wp.tile([P, fd], DT.float32, tag="mh1", name=f"mh1{t}")
            ts_eng.tensor_scalar(mh1[:], mhi[:], 1.0, None, OP.subtract)
            ww = wp.tile([P, fd], DT.float32, tag="w", name=f"w{t}")
            nc.vector.tensor_tensor(ww[:], uu[:], mh1[:], OP.add)
            r5 = wp.tile([P, fd], DT.float32, tag="r5", name=f"r5{t}")
            nc.vector.scalar_tensor_tensor(r5[:], ww[:], float(-C_HI), tt[:], OP.mult, OP.add)
            wcl = wp.tile([P, fd], DT.float32, tag="wcl", name=f"wcl{t}")
            ts_eng.tensor_scalar(wcl[:], ww[:], float(C_LO), None, OP.mult)
            rr = rp.tile([P, fd], DT.float32, tag="r", name=f"r{t}")
            nc.vector.tensor_tensor(rr[:], r5[:], wcl[:], OP.subtract)
            # rr = mod(x + pi, 2pi); the -pi shift is folded into tanh bias.

            psums = [
                pp.tile([P, CHUNK], DT.float32, tag="ps", name=f"ps{t}_{c}")
                for c in range(nch)
            ]
            acc = None
            n_gp = HID - pe_js
            for j in range(HID):
                hj = hp.tile([P, fd], h_dt if j < pe_js else DT.float32,
                             tag="h", name=f"h{t}_{j}")
                scale = float(w1[j, 0])
                nc.scalar.activation(hj[:], rr[:], AF.Tanh, bias=biases[j][:], scale=scale)
                if j < pe_js:
                    for c in range(nch):
                        nc.tensor.matmul(
                            psums[c][:],
                            mm_ap(diags[j][:]),
                            mm_ap(hj[:, c * CHUNK:(c + 1) * CHUNK]),
                            start=(j == 0),
                            stop=(j == pe_js - 1),
                        )
                elif acc is None:
                    acc = rp.tile([P, fd], DT.float32, tag="acc", name=f"acc{t}")
                    nc.gpsimd.tensor_scalar(
                        acc[:], hj[:], float(w2[0, j]), float(b2[0]), OP.mult, OP.add
                    )
                else:
                    nc.gpsimd.scalar_tensor_tensor(
                        acc[:], hj[:], float(w2[0, j]), acc[:], OP.mult, OP.add
                    )

            ot = op_.tile([P, fd], DT.float32, tag="o", name=f"o{t}")
            for c in range(nch):
                sl = slice(c * CHUNK, (c + 1) * CHUNK)
                if n_gp == 0:
                    nc.vector.tensor_scalar(ot[:, sl], psums[c][:], float(b2[0]), None, OP.add)
                else:
                    nc.vector.tensor_tensor(ot[:, sl], psums[c][:], acc[:, sl], OP.add)
            nc.sync.dma_start(y_dst, ot[:])


PE_JS = HID


def build_nc(w1, b1, w2, b2, n_core=N_CORE, fd_list=FD_LIST, mm_mode=MM_MODE,
             gp_offload=GP_OFFLOAD, pe_js=None):
    if pe_js is None:
        pe_js = PE_JS
    assert sum(fd_list) * P == n_core, (fd_list, n_core)
    assert all(fd % CHUNK == 0 for fd in fd_list)
    nc = bacc.Bacc("TRN2", target_bir_lowering=False, debug=False)
    x = nc.dram_tensor("x", [n_core], DT.float32, kind="ExternalInput")
    y = nc.dram_tensor("y", [n_core], DT.float32, kind="ExternalOutput")
    with tile.TileContext(nc) as tc:
        emit(nc, tc, x, y, w1, b1, w2, b2, fd_list, mm_mode, gp_offload, pe_js)
    nc.compile()
    return nc


@functools.lru_cache(maxsize=4)
def _built(weight_bytes, n_core, fd_list):
    w1, b1, w2, b2 = _unpack_weights(weight_bytes)
    return build_nc(w1, b1, w2, b2, n_core, fd_list)


def _pack_weights(w1, b1, w2, b2):
    return (
        w1.astype(f32).tobytes()
        + b1.astype(f32).tobytes()
        + w2.astype(f32).tobytes()
        + b2.astype(f32).tobytes()
    )


def _unpack_weights(buf):
    a = np.frombuffer(buf, dtype=f32)
    return (
        a[0:10].reshape(10, 1),
        a[10:20].reshape(10),
        a[20:30].reshape(1, 10),
        a[30:31].reshape(1),
    )


def kernel(x, w1, b1, w2, b2, _trace=False, _trace_kwargs=None):
    x = np.ascontiguousarray(x, dtype=f32)
    n = x.shape[0]
    assert x.size == n, "x must be [N, 1] or [N]"
    assert n % N_CORES == 0
    n_core = n // N_CORES
    if sum(FD_LIST) * P == n_core:
        fd_list = FD_LIST
    else:
        assert n_core % (P * CHUNK) == 0
        fd_list = (CHUNK,) * (n_core // (P * CHUNK))

    nc = _built(_pack_weights(w1, b1, w2, b2), n_core, fd_list)

    xf = x.reshape(-1)
    in_maps = [
        {"x": xf[c * n_core:(c + 1) * n_core]}
        for c in range(N_CORES)
    ]
    try:
        res = run_bass_kernel_spmd(
            nc,
            in_maps,
            core_ids=list(range(N_CORES)),
            trace=_trace,
            **(_trace_kwargs or {}),
        )
    except (ImportError, ModuleNotFoundError):
        # NTFF profiling hook unavailable in this environment — run untraced.
        res = run_bass_kernel_spmd(
            nc, in_maps, core_ids=list(range(N_CORES)), trace=False,
        )
    out = np.concatenate([res.results[c]["y"].reshape(-1) for c in range(N_CORES)])
    out = out.reshape(x.shape).astype(f32, copy=False)
    if _trace:
        kernel._last_results = res
    return out


# revision 22
# speedup vs baseline: 1.1549x; 1.1549x over previous
"""Trainium2 Bass kernel for nn_NeuralNetwork_89833535963626.

Computes, for x of shape [N, 1] and a tiny 1-10-1 MLP:
    xw  = mod(x + pi, 2*pi) - pi
    out = tanh(xw @ w1.T + b1) @ w2.T + b2

Sharding: pure data parallel over 8 NeuronCores (batch split).
Weights are baked into the instruction stream at build time.

Per-core pipeline (all engines concurrent under Tile):
  DVE : angle wrap via magic-number rounding + split-constant residual;
        the wrap-count decision is made with Sterbenz-exact quantities so
        boundary elements match IEEE f32 floored-mod bit-for-bit.
  ACT : 10x tanh(scale*r + bias) with w1/b1 (and the -pi shift) folded
        into the activation's free affine.
  PE  : weighted sum over the 10 hidden units as diag(w2_j) matmuls
        accumulated in PSUM (fp32).
  DVE : PSUM -> SBUF evacuation fused with the +b2 bias.
"""
import functools
import sys

import numpy as np

for _p in ("/opt/trn_rl_repo", "/root/.axon_site", "/root/.axon_site/_ro/pypackages"):
    if _p not in sys.path:
        sys.path.append(_p)

from contextlib import ExitStack

import concourse.bass as bass
import concourse.tile as tile
from concourse import bacc, mybir
from concourse.bass_utils import run_bass_kernel_spmd

AF = mybir.ActivationFunctionType
OP = mybir.AluOpType
DT = mybir.dt

f32 = np.float32

N_TOTAL = 4194304
N_CORES = 8
N_CORE = N_TOTAL // N_CORES
P = 128
FD = 2048                  # legacy default tile free-dim
FD_LIST = (512, 1024, 1024, 1024, 512)   # per-supertile free dims; sum*P == N_CORE
CHUNK = 512                # psum / matmul chunk (one PSUM bank, fp32)
HID = 10
MM_MODE = "f32"            # "f32" | "f32r" | "bf16" — PE operand dtype
GP_OFFLOAD = False

# wrap constants
PI = f32(np.pi)
B = f32(2.0 * np.pi)
_C_HI = (np.float32(B).view(np.uint32) & np.uint32(0xFFFFE000)).view(np.float32)
C_HI = f32(_C_HI)                       # 11-bit mantissa: k*C_HI exact for |k|<=8192
C_LO = f32(np.float64(B) - np.float64(C_HI))
INV_B = f32(1.0 / np.float64(B))
MAGIC = f32(2 ** 23 + 2 ** 22)


def emit(nc, tc, x_dram, y_dram, w1, b1, w2, b2, fd_list, mm_mode=MM_MODE,
         gp_offload=True, pe_js=HID):
    h_dt = {"bf16": DT.bfloat16, "f32r": DT.float32r, "f32": DT.float32}[mm_mode]
    diag_dt = h_dt

    def mm_ap(ap):
        return ap

    # engine for the six single-input tensor_scalar ops of the wrap chain
    ts_eng = nc.gpsimd if gp_offload else nc.vector

    ctx = ExitStack()
    with ctx:
        const = ctx.enter_context(tc.tile_pool(name="const", bufs=1))
        xp = ctx.enter_context(tc.tile_pool(name="xp", bufs=2))
        wp = ctx.enter_context(tc.tile_pool(name="wrap", bufs=1))
        rp = ctx.enter_context(tc.tile_pool(name="rp", bufs=2))
        hp = ctx.enter_context(tc.tile_pool(name="hp", bufs=3))
        op_ = ctx.enter_context(tc.tile_pool(name="op", bufs=2))
        pp = ctx.enter_context(tc.tile_pool(name="pp", bufs=8, space="PSUM"))

        # diag(w2_j) stationary tiles, synthesized on-chip
        iota_t = const.tile([P, P], DT.int32, tag="iota", name="iota_t")
        nc.gpsimd.iota(iota_t[:], pattern=[[1, P]], base=0, channel_multiplier=-1)
        ident = const.tile([P, P], DT.float32, tag="ident", name="ident")
        nc.vector.tensor_scalar(ident[:], iota_t[:], 0, None, OP.is_equal)
        diags = []
        for j in range(HID):
            dj = const.tile([P, P], diag_dt, tag=f"diag{j}", name=f"diag{j}")
            nc.vector.tensor_scalar(dj[:], ident[:], float(w2[0, j]), None, OP.mult)
            diags.append(dj)
        biases = []
        for j in range(HID):
            bval = float(f32(np.float64(b1[j]) - np.float64(w1[j, 0]) * np.float64(PI)))
            bt = const.tile([P, 1], DT.float32, tag=f"b{j}", name=f"bias{j}")
            nc.gpsimd.memset(bt[:], bval)
            biases.append(bt)

        x_flat = x_dram.ap()
        y_flat = y_dram.ap()

        off = 0
        for t, fd in enumerate(fd_list):
            nch = fd // CHUNK
            x_src = x_flat[off:off + P * fd].rearrange("(p f) -> p f", f=fd)
            y_dst = y_flat[off:off + P * fd].rearrange("(p f) -> p f", f=fd)
            off += P * fd

            xt = xp.tile([P, fd], DT.float32, tag="x", name=f"x{t}")
            nc.sync.dma_start(xt[:], x_src)

            tt = wp.tile([P, fd], DT.float32, tag="t", name=f"t{t}")
            ts_eng.tensor_scalar(tt[:], xt[:], float(PI), None, OP.add)
            km = wp.tile([P, fd], DT.float32, tag="km", name=f"km{t}")
            ts_eng.tensor_scalar(km[:], xt[:], float(INV_B), 0.5, OP.mult, OP.add)
            uu = wp.tile([P, fd], DT.float32, tag="u", name=f"u{t}")
            ts_eng.tensor_scalar(uu[:], km[:], float(MAGIC), float(MAGIC), OP.add, OP.subtract)
            s1 = wp.tile([P, fd], DT.float32, tag="s1", name=f"s1{t}")
            nc.vector.scalar_tensor_tensor(s1[:], uu[:], float(-C_HI), tt[:], OP.mult, OP.add)
            ucl = wp.tile([P, fd], DT.float32, tag="ucl", name=f"ucl{t}")
            ts_eng.tensor_scalar(ucl[:], uu[:], float(C_LO), None, OP.mult)
            mhi = wp.tile([P, fd], DT.float32, tag="mhi", name=f"mhi{t}")
            nc.vector.tensor_tensor(mhi[:], s1[:], ucl[:], OP.is_ge)
            mh1 = wp.tile([P, fd], DT.float32, tag="mh1", name=f"mh1{t}")
            ts_eng.tensor_scalar(mh1[:], mhi[:], 1.0, None, OP.subtract)
            ww = wp.tile([P, fd], DT.float32, tag="w", name=f"w{t}")
            nc.vector.tensor_tensor(ww[:], uu[:], mh1[:], OP.add)
            r5 = wp.tile([P, fd], DT.float32, tag="r5", name=f"r5{t}")
            nc.vector.scalar_tensor_tensor(r5[:], ww[:], float(-C_HI), tt[:], OP.mult, OP.add)
            wcl = wp.tile([P, fd], DT.float32, tag="wcl", name=f"wcl{t}")
            ts_eng.tensor_scalar(wcl[:], ww[:], float(C_LO), None, OP.mult)
            rr = rp.tile([P, fd], DT.float32, tag="r", name=f"r{t}")
            nc.vector.tensor_tensor(rr[:], r5[:], wcl[:], OP.subtract)
            # rr = mod(x + pi, 2pi); the -pi shift is folded into tanh bias.

            psums = [
                pp.tile([P, CHUNK], DT.float32, tag="ps", name=f"ps{t}_{c}")
                for c in range(nch)
            ]
            acc = None
            n_gp = HID - pe_js
            for j in range(HID):
                hj = hp.tile([P, fd], h_dt if j < pe_js else DT.float32,
                             tag="h", name=f"h{t}_{j}")
                scale = float(w1[j, 0])
                nc.scalar.activation(hj[:], rr[:], AF.Tanh, bias=biases[j][:], scale=scale)
                if j < pe_js:
                    for c in range(nch):
                        nc.tensor.matmul(
                            psums[c][:],
                            mm_ap(diags[j][:]),
                            mm_ap(hj[:, c * CHUNK:(c + 1) * CHUNK]),
                            start=(j == 0),
                            stop=(j == pe_js - 1),
                        )
                elif acc is None:
                    acc = rp.tile([P, fd], DT.float32, tag="acc", name=f"acc{t}")
                    nc.vector.tensor_scalar(
                        acc[:], hj[:], float(w2[0, j]), float(b2[0]), OP.mult, OP.add
                    )
                else:
                    nc.vector.scalar_tensor_tensor(
                        acc[:], hj[:], float(w2[0, j]), acc[:], OP.mult, OP.add
                    )

            ot = op_.tile([P, fd], DT.float32, tag="o", name=f"o{t}")
            for c in range(nch):
                sl = slice(c * CHUNK, (c + 1) * CHUNK)
                if n_gp == 0:
                    nc.vector.tensor_scalar(ot[:, sl], psums[c][:], float(b2[0]), None, OP.add)
                else:
                    nc.vector.tensor_tensor(ot[:, sl], psums[c][:], acc[:, sl], OP.add)
            nc.sync.dma_start(y_dst, ot[:])


PE_JS = 7


def build_nc(w1, b1, w2, b2, n_core=N_CORE, fd_list=FD_LIST, mm_mode=MM_MODE,
             gp_offload=GP_OFFLOAD, pe_js=None):
    if pe_js is None:
        pe_js = PE_JS
    assert sum(fd_list) * P == n_core, (fd_list, n_core)
    assert all(fd % CHUNK == 0 for fd in fd_list)
    nc = bacc.Bacc("TRN2", target_bir_lowering=False, debug=False)
    x = nc.dram_tensor("x", [n_core], DT.float32, kind="ExternalInput")
    y = nc.dram_tensor("y", [n_core], DT.float32, kind="ExternalOutput")
    with tile.TileContext(nc) as tc:
        emit(nc, tc, x, y, w1, b1, w2, b2, fd_list, mm_mode, gp_offload, pe_js)
    nc.compile()
    return nc


@functools.lru_cache(maxsize=4)
def _built(weight_bytes, n_core, fd_list):
    w1, b1, w2, b2 = _unpack_weights(weight_bytes)
    return build_nc(w1, b1, w2, b2, n_core, fd_list)


def _pack_weights(w1, b1, w2, b2):
    return (
        w1.astype(f32).tobytes()
        + b1.astype(f32).tobytes()
        + w2.astype(f32).tobytes()
        + b2.astype(f32).tobytes()
    )


def _unpack_weights(buf):
    a = np.frombuffer(buf, dtype=f32)
    return (
        a[0:10].reshape(10, 1),
        a[10:20].reshape(10),
        a[20:30].reshape(1, 10),
        a[30:31].reshape(1),
    )


def kernel(x, w1, b1, w2, b2, _trace=False, _trace_kwargs=None):
    x = np.ascontiguousarray(x, dtype=f32)
    n = x.shape[0]
    assert x.size == n, "x must be [N, 1] or [N]"
    assert n % N_CORES == 0
    n_core = n // N_CORES
    if sum(FD_LIST) * P == n_core:
        fd_list = FD_LIST
    else:
        assert n_core % (P * CHUNK) == 0
        fd_list = (CHUNK,) * (n_core // (P * CHUNK))

    nc = _built(_pack_weights(w1, b1, w2, b2), n_core, fd_list)

    xf = x.reshape(-1)
    in_maps = [
        {"x": xf[c * n_core:(c + 1) * n_core]}
        for c in range(N_CORES)
    ]
    try:
        res = run_bass_kernel_spmd(
            nc,
            in_maps,
            core_ids=list(range(N_CORES)),
            trace=_trace,
            **(_trace_kwargs or {}),
        )
    except (ImportError, ModuleNotFoundError):
        # NTFF profiling hook unavailable in this environment — run untraced.
        res = run_bass_kernel_spmd(
            nc, in_maps, core_ids=list(range(N_CORES)), trace=False,
        )
    out = np.concatenate([res.results[c]["y"].reshape(-1) for c in range(N_CORES)])
    out = out.reshape(x.shape).astype(f32, copy=False)
    if _trace:
        kernel._last_results = res
    return out
